# revision 23
# baseline (speedup 1.0000x reference)
"""Trainium2 Bass kernel for GemNet AtomUpdateBlock (gnn_message_passing).

Computation (per reference):
    bases = basis_rad @ W_rbf              # [E, De]
    x     = m * bases                      # [E, De]
    z     = segment_sum(x, idx_atom, A)    # [A, De]
    x     = silu(z @ W_in)                 # [A, Da]
    3x residual: x = (x + silu(silu(x W1) W2)) / sqrt(2)

Distribution strategy: shard EDGES BY DESTINATION ATOM. The host bins the
atoms into 8 cores x T_ATOM tiles of <=128 atoms (balanced by edge count),
sorts/pads each tile's edges into K 128-edge groups, and each core computes
the segment-sum + atom MLP for its own atoms only. No collective needed;
outputs are disjoint atom slices.

Per 128-edge tile on device (bf16 matmuls, f32 PSUM):
    PE:  bases_psum = basis_radT_tile.T @ W_rbf     (K=16, N=512)
    DVE: x = bases_psum * m  (single fused PSUM-read multiply, bf16 out)
    PE:  z[a,:] += S.T @ x  (one matmul, N=512; S = one-hot scatter matrix
         precomputed host-side, DMA'd bf16, loaded as PE weights)
The (bases, mult, scatter) stream is software-pipelined one stage deep so
PE drains overlap the next tile's weight loads.
Epilogue per QUAD of 128-atom tiles (512 atoms, feature-major): z evac ->
4 bf16 PE transposes per subtile -> bf16 MLP matmuls at N=512, silu on
ACT, skip-adds as one fused DVE scalar_tensor_tensor per layer with
host-folded sqrt2 scaling.
"""

import math
import os
import sys

import numpy as np
import ml_dtypes

BF16 = ml_dtypes.bfloat16

P = 128
N_CORES = 8
DE, DA, DR, NH = 512, 256, 16, 3
T_ATOM = 20  # atom tiles per core (each up to 128 atoms); divisible by 4
INV_SQRT_2 = 0.7071067811865476

_NC_CACHE = {}
SILU_NATIVE = True


# ----------------------------------------------------------------------------
# Host-side packing
# ----------------------------------------------------------------------------

def _pack_layout(idx, n_atoms, n_cores, t_atom):
    E = idx.shape[0]
    n_bins = n_cores * t_atom
    counts = np.bincount(idx, minlength=n_atoms)

    order = np.argsort(-counts, kind="stable")
    n_rounds = math.ceil(n_atoms / n_bins)
    pad = n_rounds * n_bins - n_atoms
    padded = np.concatenate([order, np.full(pad, -1, dtype=order.dtype)])
    grid = padded.reshape(n_rounds, n_bins)
    grid[1::2] = grid[1::2, ::-1]  # snake-deal: balances edges and atoms
    bin_of_atom = np.empty(n_atoms, dtype=np.int64)
    slot_of_atom = np.empty(n_atoms, dtype=np.int64)
    valid = grid >= 0
    bin_idx = np.broadcast_to(np.arange(n_bins), grid.shape)
    round_idx = np.broadcast_to(np.arange(n_rounds)[:, None], grid.shape)
    bin_of_atom[grid[valid]] = bin_idx[valid]
    slot_of_atom[grid[valid]] = round_idx[valid]
    assert np.bincount(bin_of_atom, minlength=n_bins).max() <= P

    ebin = bin_of_atom[idx]
    eslot = slot_of_atom[idx]
    eorder = np.argsort(ebin * (P + 1) + eslot, kind="stable")
    ebin_sorted = ebin[eorder]
    bin_counts = np.bincount(ebin_sorted, minlength=n_bins)
    K = max(1, math.ceil(bin_counts.max() / P))
    bin_starts = np.zeros(n_bins + 1, dtype=np.int64)
    np.cumsum(bin_counts, out=bin_starts[1:])
    pos_in_bin = np.arange(E) - bin_starts[ebin_sorted]

    core_of_bin = np.arange(n_bins) // t_atom
    tile_of_bin = np.arange(n_bins) % t_atom
    return dict(
        K=K,
        eorder=eorder,
        core_of_edge=core_of_bin[ebin_sorted],
        flat_slot=tile_of_bin[ebin_sorted] * (K * P) + pos_in_bin,
        rel_of_edge=eslot[eorder].astype(np.int64),
        bin_of_atom=bin_of_atom,
        slot_of_atom=slot_of_atom,
        core_of_bin=core_of_bin,
        tile_of_bin=tile_of_bin,
    )


def _pack_weights(W_rbf, W_in, res_W1, res_W2):
    Ci, Cj = DE // P, DA // P
    Cr = DA // P
    win = W_in.reshape(Ci, P, Cj, P).transpose(1, 0, 2, 3).reshape(P, Ci * Cj * P)
    blocks = []
    c = INV_SQRT_2
    for l in range(NH):
        w1 = (res_W1[l] * (c ** l)).astype(np.float32)
        w2 = res_W2[l].astype(np.float32)
        for W in (w1, w2):
            blocks.append(
                W.reshape(Cr, P, Cr, P).transpose(1, 0, 2, 3).reshape(P, Cr * Cr * P)
            )
    wres = np.concatenate(blocks, axis=1)
    return (
        np.ascontiguousarray(W_rbf, dtype=BF16),
        np.ascontiguousarray(win, dtype=BF16),
        np.ascontiguousarray(wres, dtype=BF16),
    )


def _build_in_maps(m, basis_rad, layout, W_rbf, W_in, res_W1, res_W2, n_cores, t_atom):
    K = layout["K"]
    cap = t_atom * K * P
    ncols = t_atom * K
    eorder = layout["eorder"]
    core_of_edge = layout["core_of_edge"]
    flat_slot = layout["flat_slot"]
    rel = layout["rel_of_edge"]

    wrbf, win, wres = _pack_weights(W_rbf, W_in, res_W1, res_W2)
    m_src = m[eorder]
    bas_src = basis_rad[eorder]
    ident = np.eye(P, dtype=np.float32)

    in_maps = []
    for c in range(n_cores):
        sel = core_of_edge == c
        fs = flat_slot[sel]
        m_pack = np.zeros((cap, DE), dtype=BF16)
        m_pack[fs] = m_src[sel].astype(BF16)
        # partition-major: m2[p, col*DE + d] = m_pack[col*P + p, d] so each
        # partition's per-atom-tile DMA read is fully contiguous
        m_pack = np.ascontiguousarray(
            m_pack.reshape(ncols, P, DE).transpose(1, 0, 2).reshape(P, ncols * DE)
        )
        basT = np.zeros((DR, cap), dtype=BF16)
        basT[:, fs] = bas_src[sel].T.astype(BF16)
        rel_flat = np.full(cap, -1, dtype=np.int64)
        rel_flat[fs] = rel[sel]
        rel2 = rel_flat.reshape(ncols, P).T  # [p, col]
        s_host = (rel2[:, :, None] == np.arange(P)[None, None, :]).astype(BF16)
        in_maps.append(
            dict(
                m_pack=m_pack,
                basT=np.ascontiguousarray(basT),
                s_hot=np.ascontiguousarray(s_host.reshape(P, ncols * P)),
                wrbf=wrbf,
                win=win,
                wres=wres,
                ident=ident,
            )
        )
    return in_maps


def _unpack_output(results, layout, n_atoms, n_cores, t_atom):
    Cj = DA // P
    out = np.zeros((n_atoms, DA), dtype=np.float32)
    core_of_atom = layout["core_of_bin"][layout["bin_of_atom"]]
    row_of_atom = (
        layout["tile_of_bin"][layout["bin_of_atom"]] * P + layout["slot_of_atom"]
    )
    for c in range(n_cores):
        x = results[c]["out"].reshape(P, Cj, t_atom, P)
        x_core = x.transpose(2, 3, 1, 0).reshape(t_atom * P, DA)
        mask = core_of_atom == c
        out[mask] = x_core[row_of_atom[mask]]
    return out


# ----------------------------------------------------------------------------
# Bass kernel builder
# ----------------------------------------------------------------------------

def _build_nc(t_atom, K):
    import concourse.mybir as mybir
    import concourse.tile as tile
    from concourse import bacc

    f32 = mybir.dt.float32
    bf16 = mybir.dt.bfloat16
    Ci, Cj = DE // P, DA // P
    Cr = DA // P
    cap = t_atom * K * P
    ncols = t_atom * K
    C3 = INV_SQRT_2 ** NH
    GAMMA = [float((1.0 / INV_SQRT_2) ** l) for l in range(NH)]
    assert t_atom % 4 == 0
    n_quads = t_atom // 4
    W4 = 4 * P  # atoms per epilogue quad

    nc = bacc.Bacc(
        "TRN2",
        target_bir_lowering=False,
        debug=False,
        enable_asserts=False,
        num_devices=N_CORES,
    )
    d_m = nc.dram_tensor("m_pack", [P, ncols * DE], bf16, kind="ExternalInput")
    d_basT = nc.dram_tensor("basT", [DR, cap], bf16, kind="ExternalInput")
    d_s = nc.dram_tensor("s_hot", [P, ncols * P], bf16, kind="ExternalInput")
    d_wrbf = nc.dram_tensor("wrbf", [DR, DE], bf16, kind="ExternalInput")
    d_win = nc.dram_tensor("win", [P, Ci * Cj * P], bf16, kind="ExternalInput")
    d_wres = nc.dram_tensor(
        "wres", [P, NH * 2 * Cr * Cr * P], bf16, kind="ExternalInput"
    )
    d_ident = nc.dram_tensor("ident", [P, P], f32, kind="ExternalInput")
    d_out = nc.dram_tensor("out", [P, Cj * t_atom * P], f32, kind="ExternalOutput")

    with tile.TileContext(nc) as tc:
        with (
            tc.tile_pool(name="const", bufs=1) as const_p,
            tc.tile_pool(name="bas", bufs=2) as bas_p,
            tc.tile_pool(name="m", bufs=2) as m_p,
            tc.tile_pool(name="x", bufs=4) as x_p,
            tc.tile_pool(name="s", bufs=2) as s_p,
            tc.tile_pool(name="zsb", bufs=2) as zsb_p,
            tc.tile_pool(name="ztsb", bufs=2) as ztsb_p,
            tc.tile_pool(name="act", bufs=3) as act_p,
            tc.tile_pool(name="outp", bufs=2) as out_p,
            tc.tile_pool(name="ps_bases", bufs=3, space="PSUM") as psb_p,
            tc.tile_pool(name="ps_z", bufs=2, space="PSUM") as psz_p,
            tc.tile_pool(name="ps_misc", bufs=3, space="PSUM") as psm_p,
        ):
            _ctr = [0]

            def emit_silu(out_ap, in_ps_ap):
                if SILU_NATIVE:
                    nc.scalar.activation(
                        out=out_ap, in_=in_ps_ap,
                        func=mybir.ActivationFunctionType.Silu,
                    )
                else:
                    _ctr[0] += 1
                    sg = act_p.tile([P, W4], f32, tag="sig", name=f"sig{_ctr[0]}")
                    nc.scalar.activation(
                        out=sg[:], in_=in_ps_ap,
                        func=mybir.ActivationFunctionType.Sigmoid,
                    )
                    nc.vector.tensor_tensor(
                        out=out_ap, in0=in_ps_ap, in1=sg[:],
                        op=mybir.AluOpType.mult,
                    )

            # Resident constants
            wrbf_sb = const_p.tile([DR, DE], bf16, tag="wrbf")
            nc.sync.dma_start(out=wrbf_sb[:], in_=d_wrbf[:])
            win_sb = const_p.tile([P, Ci * Cj * P], bf16, tag="win")
            nc.sync.dma_start(out=win_sb[:], in_=d_win[:])
            wres_sb = const_p.tile([P, NH * 2 * Cr * Cr * P], bf16, tag="wres")
            nc.sync.dma_start(out=wres_sb[:], in_=d_wres[:])
            ident = const_p.tile([P, P], f32, tag="ident")
            nc.sync.dma_start(out=ident[:], in_=d_ident[:])

            for q in range(n_quads):
                # --- scatter phase over 4 subtiles, software-pipelined ---
                subs = []  # (bas_sb, m_t, s_t, z_ps) per subtile
                for sub in range(4):
                    t = 4 * q + sub
                    bas_sb = bas_p.tile([DR, K * P], bf16, tag="bas")
                    nc.sync.dma_start(
                        out=bas_sb[:], in_=d_basT[:, t * K * P : (t + 1) * K * P]
                    )
                    m_t = m_p.tile([P, K * DE], bf16, tag="m")
                    nc.sync.dma_start(
                        out=m_t[:], in_=d_m[:, t * K * DE : (t + 1) * K * DE]
                    )
                    s_t = s_p.tile([P, K * P], bf16, tag="s")
                    nc.sync.dma_start(
                        out=s_t[:], in_=d_s[:, t * K * P : (t + 1) * K * P]
                    )
                    z_ps = psz_p.tile(
                        [P, DE], f32, space="PSUM", tag="z", name=f"zps{t}"
                    )
                    subs.append((bas_sb, m_t, s_t, z_ps))

                def do_evac(sub):
                    """z psum -> sbuf -> 4 bf16 transposes -> zt_sb columns."""
                    t = 4 * q + sub
                    z_sb = zsb_p.tile([P, DE], f32, tag="zsb", name=f"zsb{t}")
                    nc.scalar.copy(out=z_sb[:], in_=subs[sub][3][:])
                    for c in range(Ci):
                        zt_ps = psm_p.tile(
                            [P, P], f32, space="PSUM", tag="misc", name=f"ztp{t}_{c}"
                        )
                        nc.tensor.transpose(
                            out=zt_ps[:],
                            in_=z_sb[:, c * P : (c + 1) * P],
                            identity=ident[:],
                        )
                        nc.scalar.copy(
                            out=zt_sb[:, c * W4 + sub * P : c * W4 + (sub + 1) * P],
                            in_=zt_ps[:],
                        )

                zt_sb = ztsb_p.tile([P, Ci * W4], bf16, tag="ztsb")
                cols = [(sub, k) for sub in range(4) for k in range(K)]
                pend = None  # (sub, k, b_ps) awaiting mult+scatter
                for sub, k in cols:
                    bas_sb, m_t, s_t, z_ps = subs[sub]
                    b_ps = psb_p.tile([P, DE], f32, space="PSUM", tag="bases")
                    nc.tensor.matmul(
                        out=b_ps[:],
                        lhsT=bas_sb[:, k * P : (k + 1) * P],
                        rhs=wrbf_sb[:],
                        start=True,
                        stop=True,
                    )
                    if pend is not None:
                        psub, pk, pb = pend
                        pbas, pm, ps, pz = subs[psub]
                        x_t = x_p.tile([P, DE], bf16, tag="x")
                        nc.vector.tensor_tensor(
                            out=x_t[:],
                            in0=pb[:],
                            in1=pm[:, pk * DE : (pk + 1) * DE],
                            op=mybir.AluOpType.mult,
                        )
                        nc.tensor.matmul(
                            out=pz[:],
                            lhsT=ps[:, pk * P : (pk + 1) * P],
                            rhs=x_t[:],
                            start=(pk == 0),
                            stop=(pk == K - 1),
                        )
                        if pk == K - 1:
                            do_evac(psub)
                    pend = (sub, k, b_ps)
                # drain the last pending column
                psub, pk, pb = pend
                pbas, pm, ps, pz = subs[psub]
                x_t = x_p.tile([P, DE], bf16, tag="x")
                nc.vector.tensor_tensor(
                    out=x_t[:],
                    in0=pb[:],
                    in1=pm[:, pk * DE : (pk + 1) * DE],
                    op=mybir.AluOpType.mult,
                )
                nc.tensor.matmul(
                    out=pz[:],
                    lhsT=ps[:, pk * P : (pk + 1) * P],
                    rhs=x_t[:],
                    start=(pk == 0),
                    stop=(pk == K - 1),
                )
                do_evac(psub)

                # ---- quad epilogue (512 atoms, feature-major, bf16) ----
                u_ps = [
                    psm_p.tile(
                        [P, W4], f32, space="PSUM", tag="misc", name=f"ups{q}_{j}"
                    )
                    for j in range(Cj)
                ]
                for j in range(Cj):
                    for c in range(Ci):
                        fi = c * Cj + j
                        nc.tensor.matmul(
                            out=u_ps[j][:],
                            lhsT=win_sb[:, fi * P : (fi + 1) * P],
                            rhs=zt_sb[:, c * W4 : (c + 1) * W4],
                            start=(c == 0),
                            stop=(c == Ci - 1),
                        )
                X = act_p.tile([P, Cr * W4], bf16, tag="X", name=f"X{q}_0")
                for j in range(Cj):
                    emit_silu(X[:, j * W4 : (j + 1) * W4], u_ps[j][:])
                for l in range(NH):
                    v_ps = [
                        psm_p.tile(
                            [P, W4], f32, space="PSUM", tag="misc", name=f"vps{q}_{l}_{j}"
                        )
                        for j in range(Cr)
                    ]
                    for j in range(Cr):
                        for i in range(Cr):
                            fi = ((l * 2 + 0) * Cr + i) * Cr + j
                            nc.tensor.matmul(
                                out=v_ps[j][:],
                                lhsT=wres_sb[:, fi * P : (fi + 1) * P],
                                rhs=X[:, i * W4 : (i + 1) * W4],
                                start=(i == 0),
                                stop=(i == Cr - 1),
                            )
                    u1 = act_p.tile([P, Cr * W4], bf16, tag="u1", name=f"u1_{q}_{l}")
                    for j in range(Cr):
                        emit_silu(u1[:, j * W4 : (j + 1) * W4], v_ps[j][:])
                    w_ps = [
                        psm_p.tile(
                            [P, W4], f32, space="PSUM", tag="misc", name=f"wps{q}_{l}_{j}"
                        )
                        for j in range(Cr)
                    ]
                    for j in range(Cr):
                        for i in range(Cr):
                            fi = ((l * 2 + 1) * Cr + i) * Cr + j
                            nc.tensor.matmul(
                                out=w_ps[j][:],
                                lhsT=wres_sb[:, fi * P : (fi + 1) * P],
                                rhs=u1[:, i * W4 : (i + 1) * W4],
                                start=(i == 0),
                                stop=(i == Cr - 1),
                            )
                    Y = act_p.tile([P, Cr * W4], bf16, tag="y", name=f"Y{q}_{l}")
                    for j in range(Cr):
                        emit_silu(Y[:, j * W4 : (j + 1) * W4], w_ps[j][:])
                    Xn = act_p.tile([P, Cr * W4], bf16, tag="X", name=f"X{q}_{l + 1}")
                    nc.vector.scalar_tensor_tensor(
                        out=Xn[:],
                        in0=Y[:],
                        scalar=GAMMA[l],
                        in1=X[:],
                        op0=mybir.AluOpType.mult,
                        op1=mybir.AluOpType.add,
                    )
                    X = Xn
                o_t = out_p.tile([P, Cj * W4], f32, tag="out")
                nc.scalar.mul(out=o_t[:], in_=X[:], mul=float(C3))
                for j in range(Cj):
                    nc.sync.dma_start(
                        out=d_out[
                            :, (j * t_atom + 4 * q) * P : (j * t_atom + 4 * q + 4) * P
                        ],
                        in_=o_t[:, j * W4 : (j + 1) * W4],
                    )
    nc.compile()
    return nc


def _get_nc(t_atom, K):
    key = (t_atom, K)
    if key not in _NC_CACHE:
        _NC_CACHE[key] = _build_nc(t_atom, K)
    return _NC_CACHE[key]


# ----------------------------------------------------------------------------
# Entry point
# ----------------------------------------------------------------------------

def kernel(h, m, basis_rad, idx_atom, W_rbf, W_in, res_W1, res_W2):
    from concourse.bass_utils import run_bass_kernel_spmd

    m = np.asarray(m, dtype=np.float32)
    basis_rad = np.asarray(basis_rad, dtype=np.float32)
    idx = np.asarray(idx_atom).astype(np.int64)
    W_rbf = np.asarray(W_rbf, dtype=np.float32)
    W_in = np.asarray(W_in, dtype=np.float32)
    res_W1 = np.asarray(res_W1, dtype=np.float32)
    res_W2 = np.asarray(res_W2, dtype=np.float32)
    n_atoms = np.asarray(h).shape[0]

    layout = _pack_layout(idx, n_atoms, N_CORES, T_ATOM)
    in_maps = _build_in_maps(
        m, basis_rad, layout, W_rbf, W_in, res_W1, res_W2, N_CORES, T_ATOM
    )
    nc = _get_nc(T_ATOM, layout["K"])

    trace = os.environ.get("KERNEL_TRACE", "0") == "1"
    res = run_bass_kernel_spmd(
        nc, in_maps, core_ids=list(range(N_CORES)), trace=trace
    )
    if trace and res.exec_time_ns is not None:
        print(f"HW exec time: {res.exec_time_ns} ns", file=sys.stderr)
        kernel.last_exec_time_ns = res.exec_time_ns
    kernel.last_results = res
    return _unpack_output(res.results, layout, n_atoms, N_CORES, T_ATOM)


# revision 24
# speedup vs baseline: 1.1995x; 1.1995x over previous
"""Trainium2 Bass kernel for GemNet AtomUpdateBlock (gnn_message_passing).

Computation (per reference):
    bases = basis_rad @ W_rbf              # [E, De]
    x     = m * bases                      # [E, De]
    z     = segment_sum(x, idx_atom, A)    # [A, De]
    x     = silu(z @ W_in)                 # [A, Da]
    3x residual: x = (x + silu(silu(x W1) W2)) / sqrt(2)

Distribution strategy: shard EDGES BY DESTINATION ATOM. The host bins the
atoms into 8 cores x T_ATOM tiles of <=128 atoms (balanced by edge count),
sorts/pads each tile's edges into K 128-edge groups, and each core computes
the segment-sum + atom MLP for its own atoms only. No collective needed;
outputs are disjoint atom slices.

Per 128-edge tile on device (bf16 matmuls, f32 PSUM):
    PE:  bases_psum = basis_radT_tile.T @ W_rbf     (K=16, N=512)
    DVE: x = bases_psum * m  (single fused PSUM-read multiply, bf16 out)
    PE:  z[a,:] += S.T @ x  (one matmul, N=512; S = one-hot scatter matrix
         precomputed host-side, DMA'd bf16, loaded as PE weights)
The (bases, mult, scatter) stream is software-pipelined one stage deep so
PE drains overlap the next tile's weight loads.
Epilogue per QUAD of 128-atom tiles (512 atoms, feature-major): z evac ->
4 bf16 PE transposes per subtile -> bf16 MLP matmuls at N=512, silu on
ACT, skip-adds as one fused DVE scalar_tensor_tensor per layer with
host-folded sqrt2 scaling.
"""

import math
import os
import sys

import numpy as np
import ml_dtypes

BF16 = ml_dtypes.bfloat16

P = 128
N_CORES = 8
DE, DA, DR, NH = 512, 256, 16, 3
T_ATOM = 20  # atom tiles per core (each up to 128 atoms); divisible by 4
INV_SQRT_2 = 0.7071067811865476

_NC_CACHE = {}
SILU_NATIVE = True


# ----------------------------------------------------------------------------
# Host-side packing
# ----------------------------------------------------------------------------

def _pack_layout(idx, n_atoms, n_cores, t_atom):
    E = idx.shape[0]
    n_bins = n_cores * t_atom
    counts = np.bincount(idx, minlength=n_atoms)

    order = np.argsort(-counts, kind="stable")
    n_rounds = math.ceil(n_atoms / n_bins)
    pad = n_rounds * n_bins - n_atoms
    padded = np.concatenate([order, np.full(pad, -1, dtype=order.dtype)])
    grid = padded.reshape(n_rounds, n_bins)
    grid[1::2] = grid[1::2, ::-1]  # snake-deal: balances edges and atoms
    bin_of_atom = np.empty(n_atoms, dtype=np.int64)
    slot_of_atom = np.empty(n_atoms, dtype=np.int64)
    valid = grid >= 0
    bin_idx = np.broadcast_to(np.arange(n_bins), grid.shape)
    round_idx = np.broadcast_to(np.arange(n_rounds)[:, None], grid.shape)
    bin_of_atom[grid[valid]] = bin_idx[valid]
    slot_of_atom[grid[valid]] = round_idx[valid]
    assert np.bincount(bin_of_atom, minlength=n_bins).max() <= P

    ebin = bin_of_atom[idx]
    eslot = slot_of_atom[idx]
    eorder = np.argsort(ebin * (P + 1) + eslot, kind="stable")
    ebin_sorted = ebin[eorder]
    bin_counts = np.bincount(ebin_sorted, minlength=n_bins)
    K = max(1, math.ceil(bin_counts.max() / P))
    bin_starts = np.zeros(n_bins + 1, dtype=np.int64)
    np.cumsum(bin_counts, out=bin_starts[1:])
    pos_in_bin = np.arange(E) - bin_starts[ebin_sorted]

    core_of_bin = np.arange(n_bins) // t_atom
    tile_of_bin = np.arange(n_bins) % t_atom
    return dict(
        K=K,
        eorder=eorder,
        core_of_edge=core_of_bin[ebin_sorted],
        flat_slot=tile_of_bin[ebin_sorted] * (K * P) + pos_in_bin,
        rel_of_edge=eslot[eorder].astype(np.int64),
        bin_of_atom=bin_of_atom,
        slot_of_atom=slot_of_atom,
        core_of_bin=core_of_bin,
        tile_of_bin=tile_of_bin,
    )


def _pack_weights(W_rbf, W_in, res_W1, res_W2):
    Ci, Cj = DE // P, DA // P
    Cr = DA // P
    win = W_in.reshape(Ci, P, Cj, P).transpose(1, 0, 2, 3).reshape(P, Ci * Cj * P)
    blocks = []
    c = INV_SQRT_2
    for l in range(NH):
        w1 = (res_W1[l] * (c ** l)).astype(np.float32)
        w2 = res_W2[l].astype(np.float32)
        for W in (w1, w2):
            blocks.append(
                W.reshape(Cr, P, Cr, P).transpose(1, 0, 2, 3).reshape(P, Cr * Cr * P)
            )
    wres = np.concatenate(blocks, axis=1)
    return (
        np.ascontiguousarray(W_rbf, dtype=BF16),
        np.ascontiguousarray(win, dtype=BF16),
        np.ascontiguousarray(wres, dtype=BF16),
    )


def _build_in_maps(m, basis_rad, layout, W_rbf, W_in, res_W1, res_W2, n_cores, t_atom):
    K = layout["K"]
    cap = t_atom * K * P
    ncols = t_atom * K
    eorder = layout["eorder"]
    core_of_edge = layout["core_of_edge"]
    flat_slot = layout["flat_slot"]
    rel = layout["rel_of_edge"]

    wrbf, win, wres = _pack_weights(W_rbf, W_in, res_W1, res_W2)
    m_src = m[eorder]
    bas_src = basis_rad[eorder]
    ident = np.eye(P, dtype=np.float32)

    in_maps = []
    for c in range(n_cores):
        sel = core_of_edge == c
        fs = flat_slot[sel]
        m_pack = np.zeros((cap, DE), dtype=BF16)
        m_pack[fs] = m_src[sel].astype(BF16)
        # partition-major: m2[p, col*DE + d] = m_pack[col*P + p, d] so each
        # partition's per-atom-tile DMA read is fully contiguous
        m_pack = np.ascontiguousarray(
            m_pack.reshape(ncols, P, DE).transpose(1, 0, 2).reshape(P, ncols * DE)
        )
        basT = np.zeros((DR, cap), dtype=BF16)
        basT[:, fs] = bas_src[sel].T.astype(BF16)
        rel_flat = np.full(cap, -1, dtype=np.int64)
        rel_flat[fs] = rel[sel]
        rel2 = rel_flat.reshape(ncols, P).T  # [p, col]
        s_host = (rel2[:, :, None] == np.arange(P)[None, None, :]).astype(BF16)
        in_maps.append(
            dict(
                m_pack=m_pack,
                basT=np.ascontiguousarray(basT),
                s_hot=np.ascontiguousarray(s_host.reshape(P, ncols * P)),
                wrbf=wrbf,
                win=win,
                wres=wres,
                ident=ident,
            )
        )
    return in_maps


def _unpack_output(results, layout, n_atoms, n_cores, t_atom):
    Cj = DA // P
    out = np.zeros((n_atoms, DA), dtype=np.float32)
    core_of_atom = layout["core_of_bin"][layout["bin_of_atom"]]
    row_of_atom = (
        layout["tile_of_bin"][layout["bin_of_atom"]] * P + layout["slot_of_atom"]
    )
    for c in range(n_cores):
        x = results[c]["out"].reshape(P, Cj, t_atom, P)
        x_core = x.transpose(2, 3, 1, 0).reshape(t_atom * P, DA)
        mask = core_of_atom == c
        out[mask] = x_core[row_of_atom[mask]]
    return out


# ----------------------------------------------------------------------------
# Bass kernel builder
# ----------------------------------------------------------------------------

def _build_nc(t_atom, K):
    import concourse.mybir as mybir
    import concourse.tile as tile
    from concourse import bacc

    f32 = mybir.dt.float32
    bf16 = mybir.dt.bfloat16
    Ci, Cj = DE // P, DA // P
    Cr = DA // P
    cap = t_atom * K * P
    ncols = t_atom * K
    C3 = INV_SQRT_2 ** NH
    GAMMA = [float((1.0 / INV_SQRT_2) ** l) for l in range(NH)]
    assert t_atom % 4 == 0
    n_quads = t_atom // 4
    W4 = 4 * P  # atoms per epilogue quad

    nc = bacc.Bacc(
        "TRN2",
        target_bir_lowering=False,
        debug=False,
        enable_asserts=False,
        num_devices=N_CORES,
    )
    d_m = nc.dram_tensor("m_pack", [P, ncols * DE], bf16, kind="ExternalInput")
    d_basT = nc.dram_tensor("basT", [DR, cap], bf16, kind="ExternalInput")
    d_s = nc.dram_tensor("s_hot", [P, ncols * P], bf16, kind="ExternalInput")
    d_wrbf = nc.dram_tensor("wrbf", [DR, DE], bf16, kind="ExternalInput")
    d_win = nc.dram_tensor("win", [P, Ci * Cj * P], bf16, kind="ExternalInput")
    d_wres = nc.dram_tensor(
        "wres", [P, NH * 2 * Cr * Cr * P], bf16, kind="ExternalInput"
    )
    d_ident = nc.dram_tensor("ident", [P, P], f32, kind="ExternalInput")
    d_out = nc.dram_tensor("out", [P, Cj * t_atom * P], f32, kind="ExternalOutput")

    with tile.TileContext(nc) as tc:
        with (
            tc.tile_pool(name="const", bufs=1) as const_p,
            tc.tile_pool(name="bas", bufs=4) as bas_p,
            tc.tile_pool(name="m", bufs=4) as m_p,
            tc.tile_pool(name="x", bufs=4) as x_p,
            tc.tile_pool(name="s", bufs=4) as s_p,
            tc.tile_pool(name="zsb", bufs=2) as zsb_p,
            tc.tile_pool(name="ztsb", bufs=2) as ztsb_p,
            tc.tile_pool(name="act", bufs=3) as act_p,
            tc.tile_pool(name="outp", bufs=2) as out_p,
            tc.tile_pool(name="ps_bases", bufs=3, space="PSUM") as psb_p,
            tc.tile_pool(name="ps_z", bufs=2, space="PSUM") as psz_p,
            tc.tile_pool(name="ps_misc", bufs=3, space="PSUM") as psm_p,
        ):
            _ctr = [0]

            def emit_silu(out_ap, in_ps_ap):
                if SILU_NATIVE:
                    nc.scalar.activation(
                        out=out_ap, in_=in_ps_ap,
                        func=mybir.ActivationFunctionType.Silu,
                    )
                else:
                    _ctr[0] += 1
                    sg = act_p.tile([P, W4], f32, tag="sig", name=f"sig{_ctr[0]}")
                    nc.scalar.activation(
                        out=sg[:], in_=in_ps_ap,
                        func=mybir.ActivationFunctionType.Sigmoid,
                    )
                    nc.vector.tensor_tensor(
                        out=out_ap, in0=in_ps_ap, in1=sg[:],
                        op=mybir.AluOpType.mult,
                    )

            # Resident constants
            wrbf_sb = const_p.tile([DR, DE], bf16, tag="wrbf")
            nc.sync.dma_start(out=wrbf_sb[:], in_=d_wrbf[:])
            win_sb = const_p.tile([P, Ci * Cj * P], bf16, tag="win")
            nc.sync.dma_start(out=win_sb[:], in_=d_win[:])
            wres_sb = const_p.tile([P, NH * 2 * Cr * Cr * P], bf16, tag="wres")
            nc.sync.dma_start(out=wres_sb[:], in_=d_wres[:])
            ident = const_p.tile([P, P], f32, tag="ident")
            nc.sync.dma_start(out=ident[:], in_=d_ident[:])

            for q in range(n_quads):
                # --- scatter phase over 4 subtiles, software-pipelined ---
                subs = []  # (bas_sb, m_t, s_t, z_ps) per subtile
                for sub in range(4):
                    t = 4 * q + sub
                    bas_sb = bas_p.tile([DR, K * P], bf16, tag="bas")
                    nc.sync.dma_start(
                        out=bas_sb[:], in_=d_basT[:, t * K * P : (t + 1) * K * P]
                    )
                    m_t = m_p.tile([P, K * DE], bf16, tag="m")
                    nc.sync.dma_start(
                        out=m_t[:], in_=d_m[:, t * K * DE : (t + 1) * K * DE]
                    )
                    s_t = s_p.tile([P, K * P], bf16, tag="s")
                    nc.sync.dma_start(
                        out=s_t[:], in_=d_s[:, t * K * P : (t + 1) * K * P]
                    )
                    z_ps = psz_p.tile(
                        [P, DE], f32, space="PSUM", tag="z", name=f"zps{t}"
                    )
                    subs.append((bas_sb, m_t, s_t, z_ps))

                def do_evac(sub):
                    """z psum -> sbuf -> 4 bf16 transposes -> zt_sb columns."""
                    t = 4 * q + sub
                    z_sb = zsb_p.tile([P, DE], f32, tag="zsb", name=f"zsb{t}")
                    nc.scalar.copy(out=z_sb[:], in_=subs[sub][3][:])
                    for c in range(Ci):
                        zt_ps = psm_p.tile(
                            [P, P], f32, space="PSUM", tag="misc", name=f"ztp{t}_{c}"
                        )
                        nc.tensor.transpose(
                            out=zt_ps[:],
                            in_=z_sb[:, c * P : (c + 1) * P],
                            identity=ident[:],
                        )
                        nc.scalar.copy(
                            out=zt_sb[:, c * W4 + sub * P : c * W4 + (sub + 1) * P],
                            in_=zt_ps[:],
                        )

                zt_sb = ztsb_p.tile([P, Ci * W4], bf16, tag="ztsb")
                cols = [(sub, k) for sub in range(4) for k in range(K)]
                pend = None  # (sub, k, b_ps) awaiting mult+scatter
                for sub, k in cols:
                    bas_sb, m_t, s_t, z_ps = subs[sub]
                    b_ps = psb_p.tile([P, DE], f32, space="PSUM", tag="bases")
                    nc.tensor.matmul(
                        out=b_ps[:],
                        lhsT=bas_sb[:, k * P : (k + 1) * P],
                        rhs=wrbf_sb[:],
                        start=True,
                        stop=True,
                    )
                    if pend is not None:
                        psub, pk, pb = pend
                        pbas, pm, ps, pz = subs[psub]
                        x_t = x_p.tile([P, DE], bf16, tag="x")
                        nc.vector.tensor_tensor(
                            out=x_t[:],
                            in0=pb[:],
                            in1=pm[:, pk * DE : (pk + 1) * DE],
                            op=mybir.AluOpType.mult,
                        )
                        nc.tensor.matmul(
                            out=pz[:],
                            lhsT=ps[:, pk * P : (pk + 1) * P],
                            rhs=x_t[:],
                            start=(pk == 0),
                            stop=(pk == K - 1),
                        )
                        if pk == K - 1:
                            do_evac(psub)
                    pend = (sub, k, b_ps)
                # drain the last pending column
                psub, pk, pb = pend
                pbas, pm, ps, pz = subs[psub]
                x_t = x_p.tile([P, DE], bf16, tag="x")
                nc.vector.tensor_tensor(
                    out=x_t[:],
                    in0=pb[:],
                    in1=pm[:, pk * DE : (pk + 1) * DE],
                    op=mybir.AluOpType.mult,
                )
                nc.tensor.matmul(
                    out=pz[:],
                    lhsT=ps[:, pk * P : (pk + 1) * P],
                    rhs=x_t[:],
                    start=(pk == 0),
                    stop=(pk == K - 1),
                )
                do_evac(psub)

                # ---- quad epilogue (512 atoms, feature-major, bf16) ----
                u_ps = [
                    psm_p.tile(
                        [P, W4], f32, space="PSUM", tag="misc", name=f"ups{q}_{j}"
                    )
                    for j in range(Cj)
                ]
                for j in range(Cj):
                    for c in range(Ci):
                        fi = c * Cj + j
                        nc.tensor.matmul(
                            out=u_ps[j][:],
                            lhsT=win_sb[:, fi * P : (fi + 1) * P],
                            rhs=zt_sb[:, c * W4 : (c + 1) * W4],
                            start=(c == 0),
                            stop=(c == Ci - 1),
                        )
                X = act_p.tile([P, Cr * W4], bf16, tag="X", name=f"X{q}_0")
                for j in range(Cj):
                    emit_silu(X[:, j * W4 : (j + 1) * W4], u_ps[j][:])
                for l in range(NH):
                    v_ps = [
                        psm_p.tile(
                            [P, W4], f32, space="PSUM", tag="misc", name=f"vps{q}_{l}_{j}"
                        )
                        for j in range(Cr)
                    ]
                    for j in range(Cr):
                        for i in range(Cr):
                            fi = ((l * 2 + 0) * Cr + i) * Cr + j
                            nc.tensor.matmul(
                                out=v_ps[j][:],
                                lhsT=wres_sb[:, fi * P : (fi + 1) * P],
                                rhs=X[:, i * W4 : (i + 1) * W4],
                                start=(i == 0),
                                stop=(i == Cr - 1),
                            )
                    u1 = act_p.tile([P, Cr * W4], bf16, tag="u1", name=f"u1_{q}_{l}")
                    for j in range(Cr):
                        emit_silu(u1[:, j * W4 : (j + 1) * W4], v_ps[j][:])
                    w_ps = [
                        psm_p.tile(
                            [P, W4], f32, space="PSUM", tag="misc", name=f"wps{q}_{l}_{j}"
                        )
                        for j in range(Cr)
                    ]
                    for j in range(Cr):
                        for i in range(Cr):
                            fi = ((l * 2 + 1) * Cr + i) * Cr + j
                            nc.tensor.matmul(
                                out=w_ps[j][:],
                                lhsT=wres_sb[:, fi * P : (fi + 1) * P],
                                rhs=u1[:, i * W4 : (i + 1) * W4],
                                start=(i == 0),
                                stop=(i == Cr - 1),
                            )
                    Y = act_p.tile([P, Cr * W4], bf16, tag="y", name=f"Y{q}_{l}")
                    for j in range(Cr):
                        emit_silu(Y[:, j * W4 : (j + 1) * W4], w_ps[j][:])
                    Xn = act_p.tile([P, Cr * W4], bf16, tag="X", name=f"X{q}_{l + 1}")
                    nc.vector.scalar_tensor_tensor(
                        out=Xn[:],
                        in0=Y[:],
                        scalar=GAMMA[l],
                        in1=X[:],
                        op0=mybir.AluOpType.mult,
                        op1=mybir.AluOpType.add,
                    )
                    X = Xn
                o_t = out_p.tile([P, Cj * W4], f32, tag="out")
                nc.scalar.mul(out=o_t[:], in_=X[:], mul=float(C3))
                for j in range(Cj):
                    nc.sync.dma_start(
                        out=d_out[
                            :, (j * t_atom + 4 * q) * P : (j * t_atom + 4 * q + 4) * P
                        ],
                        in_=o_t[:, j * W4 : (j + 1) * W4],
                    )
    nc.compile()
    return nc


def _get_nc(t_atom, K):
    key = (t_atom, K)
    if key not in _NC_CACHE:
        _NC_CACHE[key] = _build_nc(t_atom, K)
    return _NC_CACHE[key]


# ----------------------------------------------------------------------------
# Entry point
# ----------------------------------------------------------------------------

def kernel(h, m, basis_rad, idx_atom, W_rbf, W_in, res_W1, res_W2):
    from concourse.bass_utils import run_bass_kernel_spmd

    m = np.asarray(m, dtype=np.float32)
    basis_rad = np.asarray(basis_rad, dtype=np.float32)
    idx = np.asarray(idx_atom).astype(np.int64)
    W_rbf = np.asarray(W_rbf, dtype=np.float32)
    W_in = np.asarray(W_in, dtype=np.float32)
    res_W1 = np.asarray(res_W1, dtype=np.float32)
    res_W2 = np.asarray(res_W2, dtype=np.float32)
    n_atoms = np.asarray(h).shape[0]

    layout = _pack_layout(idx, n_atoms, N_CORES, T_ATOM)
    in_maps = _build_in_maps(
        m, basis_rad, layout, W_rbf, W_in, res_W1, res_W2, N_CORES, T_ATOM
    )
    nc = _get_nc(T_ATOM, layout["K"])

    trace = os.environ.get("KERNEL_TRACE", "0") == "1"
    res = run_bass_kernel_spmd(
        nc, in_maps, core_ids=list(range(N_CORES)), trace=trace
    )
    if trace and res.exec_time_ns is not None:
        print(f"HW exec time: {res.exec_time_ns} ns", file=sys.stderr)
        kernel.last_exec_time_ns = res.exec_time_ns
    kernel.last_results = res
    return _unpack_output(res.results, layout, n_atoms, N_CORES, T_ATOM)


# revision 32
# speedup vs baseline: 1.2060x; 1.0054x over previous
"""Trainium2 Bass kernel for GemNet AtomUpdateBlock (gnn_message_passing).

Computation (per reference):
    bases = basis_rad @ W_rbf              # [E, De]
    x     = m * bases                      # [E, De]
    z     = segment_sum(x, idx_atom, A)    # [A, De]
    x     = silu(z @ W_in)                 # [A, Da]
    3x residual: x = (x + silu(silu(x W1) W2)) / sqrt(2)

Distribution strategy: shard EDGES BY DESTINATION ATOM. The host bins the
atoms into 8 cores x T_ATOM tiles of <=128 atoms (balanced by edge count),
sorts/pads each tile's edges into K 128-edge groups, and each core computes
the segment-sum + atom MLP for its own atoms only. No collective needed;
outputs are disjoint atom slices.

Per 128-edge tile on device (bf16 matmuls, f32 PSUM):
    PE:  bases_psum = basis_radT_tile.T @ W_rbf     (K=16, N=512)
    DVE: x = bases_psum * m  (single fused PSUM-read multiply, bf16 out)
    PE:  z[a,:] += S.T @ x  (one matmul, N=512; S = one-hot scatter matrix
         precomputed host-side, DMA'd bf16, loaded as PE weights)
The (bases, mult, scatter) stream is software-pipelined two stages deep so
the PE never stalls on the DVE multiply, and the previous quad's epilogue
matmuls are interleaved into the scatter stream (keeps the PE HAM clock
gate at 8/8). A warmup matmul burst upclocks the PE while the first DMAs
stream in.
Epilogue per QUAD of 128-atom tiles (512 atoms, feature-major): z evac ->
4 f32 PE transposes per subtile -> bf16 MLP matmuls at N=512, silu on
ACT, skip-adds as one fused DVE scalar_tensor_tensor per layer with
host-folded sqrt2 scaling. Output is written feature-major [P, Cj*T*P]
f32 and untransposed on the host during unshard.
"""

import math
import os
import sys

import numpy as np
import ml_dtypes

BF16 = ml_dtypes.bfloat16

P = 128
N_CORES = 8
DE, DA, DR, NH = 512, 256, 16, 3
T_ATOM = 20  # atom tiles per core (each up to 128 atoms); divisible by 4
INV_SQRT_2 = 0.7071067811865476

_NC_CACHE = {}
SILU_NATIVE = True


# ----------------------------------------------------------------------------
# Host-side packing
# ----------------------------------------------------------------------------

def _pack_layout(idx, n_atoms, n_cores, t_atom):
    E = idx.shape[0]
    n_bins = n_cores * t_atom
    counts = np.bincount(idx, minlength=n_atoms)

    order = np.argsort(-counts, kind="stable")
    n_rounds = math.ceil(n_atoms / n_bins)
    pad = n_rounds * n_bins - n_atoms
    padded = np.concatenate([order, np.full(pad, -1, dtype=order.dtype)])
    grid = padded.reshape(n_rounds, n_bins)
    grid[1::2] = grid[1::2, ::-1]  # snake-deal: balances edges and atoms
    bin_of_atom = np.empty(n_atoms, dtype=np.int64)
    slot_of_atom = np.empty(n_atoms, dtype=np.int64)
    valid = grid >= 0
    bin_idx = np.broadcast_to(np.arange(n_bins), grid.shape)
    round_idx = np.broadcast_to(np.arange(n_rounds)[:, None], grid.shape)
    bin_of_atom[grid[valid]] = bin_idx[valid]
    slot_of_atom[grid[valid]] = round_idx[valid]
    assert np.bincount(bin_of_atom, minlength=n_bins).max() <= P

    ebin = bin_of_atom[idx]
    eslot = slot_of_atom[idx]
    eorder = np.argsort(ebin * (P + 1) + eslot, kind="stable")
    ebin_sorted = ebin[eorder]
    bin_counts = np.bincount(ebin_sorted, minlength=n_bins)
    K = max(1, math.ceil(bin_counts.max() / P))
    bin_starts = np.zeros(n_bins + 1, dtype=np.int64)
    np.cumsum(bin_counts, out=bin_starts[1:])
    pos_in_bin = np.arange(E) - bin_starts[ebin_sorted]

    core_of_bin = np.arange(n_bins) // t_atom
    tile_of_bin = np.arange(n_bins) % t_atom
    return dict(
        K=K,
        eorder=eorder,
        core_of_edge=core_of_bin[ebin_sorted],
        flat_slot=tile_of_bin[ebin_sorted] * (K * P) + pos_in_bin,
        rel_of_edge=eslot[eorder].astype(np.int64),
        bin_of_atom=bin_of_atom,
        slot_of_atom=slot_of_atom,
        core_of_bin=core_of_bin,
        tile_of_bin=tile_of_bin,
    )


def _pack_weights(W_rbf, W_in, res_W1, res_W2):
    Ci, Cj = DE // P, DA // P
    Cr = DA // P
    win = W_in.reshape(Ci, P, Cj, P).transpose(1, 0, 2, 3).reshape(P, Ci * Cj * P)
    blocks = []
    c = INV_SQRT_2
    for l in range(NH):
        w1 = (res_W1[l] * (c ** l)).astype(np.float32)
        w2 = res_W2[l].astype(np.float32)
        for W in (w1, w2):
            blocks.append(
                W.reshape(Cr, P, Cr, P).transpose(1, 0, 2, 3).reshape(P, Cr * Cr * P)
            )
    wres = np.concatenate(blocks, axis=1)
    return (
        np.ascontiguousarray(W_rbf, dtype=BF16),
        np.ascontiguousarray(win, dtype=BF16),
        np.ascontiguousarray(wres, dtype=BF16),
    )


def _build_in_maps(m, basis_rad, layout, W_rbf, W_in, res_W1, res_W2, n_cores, t_atom):
    K = layout["K"]
    cap = t_atom * K * P
    ncols = t_atom * K
    eorder = layout["eorder"]
    core_of_edge = layout["core_of_edge"]
    flat_slot = layout["flat_slot"]
    rel = layout["rel_of_edge"]

    wrbf, win, wres = _pack_weights(W_rbf, W_in, res_W1, res_W2)
    m_src = m[eorder]
    bas_src = basis_rad[eorder]
    ident = np.eye(P, dtype=np.float32)

    in_maps = []
    for c in range(n_cores):
        sel = core_of_edge == c
        fs = flat_slot[sel]
        m_pack = np.zeros((cap, DE), dtype=BF16)
        m_pack[fs] = m_src[sel].astype(BF16)
        # partition-major: m2[p, col*DE + d] = m_pack[col*P + p, d] so each
        # partition's per-atom-tile DMA read is fully contiguous
        m_pack = np.ascontiguousarray(
            m_pack.reshape(ncols, P, DE).transpose(1, 0, 2).reshape(P, ncols * DE)
        )
        basT = np.zeros((DR, cap), dtype=BF16)
        basT[:, fs] = bas_src[sel].T.astype(BF16)
        rel_flat = np.full(cap, -1, dtype=np.int64)
        rel_flat[fs] = rel[sel]
        rel2 = rel_flat.reshape(ncols, P).T  # [p, col]
        s_host = (rel2[:, :, None] == np.arange(P)[None, None, :]).astype(BF16)
        in_maps.append(
            dict(
                m_pack=m_pack,
                basT=np.ascontiguousarray(basT),
                s_hot=np.ascontiguousarray(s_host.reshape(P, ncols * P)),
                wrbf=wrbf,
                win=win,
                wres=wres,
                ident=ident,
            )
        )
    return in_maps


def _unpack_output(results, layout, n_atoms, n_cores, t_atom):
    Cj = DA // P
    out = np.zeros((n_atoms, DA), dtype=np.float32)
    core_of_atom = layout["core_of_bin"][layout["bin_of_atom"]]
    row_of_atom = (
        layout["tile_of_bin"][layout["bin_of_atom"]] * P + layout["slot_of_atom"]
    )
    for c in range(n_cores):
        x = results[c]["out"].reshape(P, Cj, t_atom, P)
        x_core = x.transpose(2, 3, 1, 0).reshape(t_atom * P, DA)
        mask = core_of_atom == c
        out[mask] = x_core[row_of_atom[mask]]
    return out


# ----------------------------------------------------------------------------
# Bass kernel builder
# ----------------------------------------------------------------------------

def _build_nc(t_atom, K):
    import concourse.mybir as mybir
    import concourse.tile as tile
    from concourse import bacc

    f32 = mybir.dt.float32
    bf16 = mybir.dt.bfloat16
    Ci, Cj = DE // P, DA // P
    Cr = DA // P
    cap = t_atom * K * P
    ncols = t_atom * K
    C3 = INV_SQRT_2 ** NH
    GAMMA = [float((1.0 / INV_SQRT_2) ** l) for l in range(NH)]
    assert t_atom % 4 == 0
    n_quads = t_atom // 4
    W4 = 4 * P  # atoms per epilogue quad

    nc = bacc.Bacc(
        "TRN2",
        target_bir_lowering=False,
        debug=False,
        enable_asserts=False,
        num_devices=N_CORES,
    )
    d_m = nc.dram_tensor("m_pack", [P, ncols * DE], bf16, kind="ExternalInput")
    d_basT = nc.dram_tensor("basT", [DR, cap], bf16, kind="ExternalInput")
    d_s = nc.dram_tensor("s_hot", [P, ncols * P], bf16, kind="ExternalInput")
    d_wrbf = nc.dram_tensor("wrbf", [DR, DE], bf16, kind="ExternalInput")
    d_win = nc.dram_tensor("win", [P, Ci * Cj * P], bf16, kind="ExternalInput")
    d_wres = nc.dram_tensor(
        "wres", [P, NH * 2 * Cr * Cr * P], bf16, kind="ExternalInput"
    )
    d_ident = nc.dram_tensor("ident", [P, P], f32, kind="ExternalInput")
    d_out = nc.dram_tensor("out", [P, Cj * t_atom * P], f32, kind="ExternalOutput")

    with tile.TileContext(nc) as tc:
        with (
            tc.tile_pool(name="const", bufs=1) as const_p,
            tc.tile_pool(name="bas", bufs=4) as bas_p,
            tc.tile_pool(name="m", bufs=4) as m_p,
            tc.tile_pool(name="x", bufs=4) as x_p,
            tc.tile_pool(name="s", bufs=4) as s_p,
            tc.tile_pool(name="zsb", bufs=2) as zsb_p,
            tc.tile_pool(name="ztsb", bufs=2) as ztsb_p,
            tc.tile_pool(name="act", bufs=3) as act_p,
            tc.tile_pool(name="outp", bufs=2) as out_p,
            tc.tile_pool(name="ps_bases", bufs=3, space="PSUM") as psb_p,
            tc.tile_pool(name="ps_z", bufs=2, space="PSUM") as psz_p,
            tc.tile_pool(name="ps_misc", bufs=3, space="PSUM") as psm_p,
        ):
            _ctr = [0]

            def emit_silu(out_ap, in_ps_ap):
                if SILU_NATIVE:
                    nc.scalar.activation(
                        out=out_ap, in_=in_ps_ap,
                        func=mybir.ActivationFunctionType.Silu,
                    )
                else:
                    _ctr[0] += 1
                    sg = act_p.tile([P, W4], f32, tag="sig", name=f"sig{_ctr[0]}")
                    nc.scalar.activation(
                        out=sg[:], in_=in_ps_ap,
                        func=mybir.ActivationFunctionType.Sigmoid,
                    )
                    nc.vector.tensor_tensor(
                        out=out_ap, in0=in_ps_ap, in1=sg[:],
                        op=mybir.AluOpType.mult,
                    )

            # Resident constants
            wrbf_sb = const_p.tile([DR, DE], bf16, tag="wrbf")
            nc.sync.dma_start(out=wrbf_sb[:], in_=d_wrbf[:])
            win_sb = const_p.tile([P, Ci * Cj * P], bf16, tag="win")
            nc.sync.dma_start(out=win_sb[:], in_=d_win[:])
            wres_sb = const_p.tile([P, NH * 2 * Cr * Cr * P], bf16, tag="wres")
            nc.sync.dma_start(out=wres_sb[:], in_=d_wres[:])
            ident = const_p.tile([P, P], f32, tag="ident")
            nc.sync.dma_start(out=ident[:], in_=d_ident[:])

            # HAM warmup: ~24 dense back-to-back matmuls on resident weights
            # upclock the PE (4/8 -> 8/8) while the first quad's DMAs stream.
            warm_ps = psm_p.tile([P, W4], f32, space="PSUM", tag="misc", name="warm")
            for w in range(24):
                nc.tensor.matmul(
                    out=warm_ps[:],
                    lhsT=win_sb[:, (w % 8) * P : (w % 8 + 1) * P],
                    rhs=win_sb[:, 0:W4],
                    start=True,
                    stop=True,
                )

            def epilogue_gen(q, zt_sb):
                """Quad epilogue emitted as units interleavable with the next
                quad's scatter stream (keeps PE continuously busy for HAM)."""
                u_ps = [
                    psm_p.tile(
                        [P, W4], f32, space="PSUM", tag="misc", name=f"ups{q}_{j}"
                    )
                    for j in range(Cj)
                ]
                for j in range(Cj):
                    for c in range(Ci):
                        fi = c * Cj + j
                        nc.tensor.matmul(
                            out=u_ps[j][:],
                            lhsT=win_sb[:, fi * P : (fi + 1) * P],
                            rhs=zt_sb[:, c * W4 : (c + 1) * W4],
                            start=(c == 0),
                            stop=(c == Ci - 1),
                        )
                        yield
                X = act_p.tile([P, Cr * W4], bf16, tag="X", name=f"X{q}_0")
                for j in range(Cj):
                    emit_silu(X[:, j * W4 : (j + 1) * W4], u_ps[j][:])
                yield
                for l in range(NH):
                    v_ps = [
                        psm_p.tile(
                            [P, W4], f32, space="PSUM", tag="misc",
                            name=f"vps{q}_{l}_{j}"
                        )
                        for j in range(Cr)
                    ]
                    for j in range(Cr):
                        for i in range(Cr):
                            fi = ((l * 2 + 0) * Cr + i) * Cr + j
                            nc.tensor.matmul(
                                out=v_ps[j][:],
                                lhsT=wres_sb[:, fi * P : (fi + 1) * P],
                                rhs=X[:, i * W4 : (i + 1) * W4],
                                start=(i == 0),
                                stop=(i == Cr - 1),
                            )
                            yield
                    u1 = act_p.tile([P, Cr * W4], bf16, tag="u1", name=f"u1_{q}_{l}")
                    for j in range(Cr):
                        emit_silu(u1[:, j * W4 : (j + 1) * W4], v_ps[j][:])
                    yield
                    w_ps = [
                        psm_p.tile(
                            [P, W4], f32, space="PSUM", tag="misc",
                            name=f"wps{q}_{l}_{j}"
                        )
                        for j in range(Cr)
                    ]
                    for j in range(Cr):
                        for i in range(Cr):
                            fi = ((l * 2 + 1) * Cr + i) * Cr + j
                            nc.tensor.matmul(
                                out=w_ps[j][:],
                                lhsT=wres_sb[:, fi * P : (fi + 1) * P],
                                rhs=u1[:, i * W4 : (i + 1) * W4],
                                start=(i == 0),
                                stop=(i == Cr - 1),
                            )
                            yield
                    Y = act_p.tile([P, Cr * W4], bf16, tag="y", name=f"Y{q}_{l}")
                    for j in range(Cr):
                        emit_silu(Y[:, j * W4 : (j + 1) * W4], w_ps[j][:])
                    yield
                    Xn = act_p.tile(
                        [P, Cr * W4], bf16, tag="X", name=f"X{q}_{l + 1}"
                    )
                    nc.vector.scalar_tensor_tensor(
                        out=Xn[:],
                        in0=Y[:],
                        scalar=GAMMA[l],
                        in1=X[:],
                        op0=mybir.AluOpType.mult,
                        op1=mybir.AluOpType.add,
                    )
                    X = Xn
                    yield
                o_t = out_p.tile([P, Cj * W4], f32, tag="out")
                nc.scalar.mul(out=o_t[:], in_=X[:], mul=float(C3))
                for j in range(Cj):
                    nc.sync.dma_start(
                        out=d_out[
                            :, (j * t_atom + 4 * q) * P : (j * t_atom + 4 * q + 4) * P
                        ],
                        in_=o_t[:, j * W4 : (j + 1) * W4],
                    )
                yield

            prev_epi = None
            for q in range(n_quads):
                # --- scatter phase over 4 subtiles, software-pipelined ---
                subs = []  # (bas_sb, m_t, s_t, z_ps) per subtile
                for sub in range(4):
                    t = 4 * q + sub
                    bas_sb = bas_p.tile([DR, K * P], bf16, tag="bas")
                    nc.sync.dma_start(
                        out=bas_sb[:], in_=d_basT[:, t * K * P : (t + 1) * K * P]
                    )
                    m_t = m_p.tile([P, K * DE], bf16, tag="m")
                    nc.sync.dma_start(
                        out=m_t[:], in_=d_m[:, t * K * DE : (t + 1) * K * DE]
                    )
                    s_t = s_p.tile([P, K * P], bf16, tag="s")
                    nc.sync.dma_start(
                        out=s_t[:], in_=d_s[:, t * K * P : (t + 1) * K * P]
                    )
                    z_ps = psz_p.tile(
                        [P, DE], f32, space="PSUM", tag="z", name=f"zps{t}"
                    )
                    subs.append((bas_sb, m_t, s_t, z_ps))

                def do_evac(sub):
                    """z psum -> sbuf -> 4 bf16 transposes -> zt_sb columns."""
                    t = 4 * q + sub
                    z_sb = zsb_p.tile([P, DE], f32, tag="zsb", name=f"zsb{t}")
                    nc.scalar.copy(out=z_sb[:], in_=subs[sub][3][:])
                    for c in range(Ci):
                        zt_ps = psm_p.tile(
                            [P, P], f32, space="PSUM", tag="misc", name=f"ztp{t}_{c}"
                        )
                        nc.tensor.transpose(
                            out=zt_ps[:],
                            in_=z_sb[:, c * P : (c + 1) * P],
                            identity=ident[:],
                        )
                        nc.scalar.copy(
                            out=zt_sb[:, c * W4 + sub * P : c * W4 + (sub + 1) * P],
                            in_=zt_ps[:],
                        )

                zt_sb = ztsb_p.tile([P, Ci * W4], bf16, tag="ztsb")
                cols = [(sub, k) for sub in range(4) for k in range(K)]

                def finish_col(pend_item):
                    psub, pk, pb = pend_item
                    pbas, pm, ps, pz = subs[psub]
                    x_t = x_p.tile([P, DE], bf16, tag="x", name=f"x{q}_{psub}_{pk}")
                    nc.vector.tensor_tensor(
                        out=x_t[:],
                        in0=pb[:],
                        in1=pm[:, pk * DE : (pk + 1) * DE],
                        op=mybir.AluOpType.mult,
                    )
                    nc.tensor.matmul(
                        out=pz[:],
                        lhsT=ps[:, pk * P : (pk + 1) * P],
                        rhs=x_t[:],
                        start=(pk == 0),
                        stop=(pk == K - 1),
                    )
                    if pk == K - 1:
                        do_evac(psub)

                pend = []  # pipeline of (sub, k, b_ps) awaiting mult+scatter
                for sub, k in cols:
                    bas_sb, m_t, s_t, z_ps = subs[sub]
                    b_ps = psb_p.tile([P, DE], f32, space="PSUM", tag="bases")
                    nc.tensor.matmul(
                        out=b_ps[:],
                        lhsT=bas_sb[:, k * P : (k + 1) * P],
                        rhs=wrbf_sb[:],
                        start=True,
                        stop=True,
                    )
                    pend.append((sub, k, b_ps))
                    if len(pend) > 2:
                        finish_col(pend.pop(0))
                    if prev_epi is not None:
                        next(prev_epi, None)
                for item in pend:
                    finish_col(item)
                if prev_epi is not None:
                    for _ in prev_epi:
                        pass
                prev_epi = epilogue_gen(q, zt_sb)
            for _ in prev_epi:
                pass

            _dead = False
            if _dead:
                # ---- quad epilogue (512 atoms, feature-major, bf16) ----
                u_ps = [
                    psm_p.tile(
                        [P, W4], f32, space="PSUM", tag="misc", name=f"ups{q}_{j}"
                    )
                    for j in range(Cj)
                ]
                for j in range(Cj):
                    for c in range(Ci):
                        fi = c * Cj + j
                        nc.tensor.matmul(
                            out=u_ps[j][:],
                            lhsT=win_sb[:, fi * P : (fi + 1) * P],
                            rhs=zt_sb[:, c * W4 : (c + 1) * W4],
                            start=(c == 0),
                            stop=(c == Ci - 1),
                        )
                X = act_p.tile([P, Cr * W4], bf16, tag="X", name=f"X{q}_0")
                for j in range(Cj):
                    emit_silu(X[:, j * W4 : (j + 1) * W4], u_ps[j][:])
                for l in range(NH):
                    v_ps = [
                        psm_p.tile(
                            [P, W4], f32, space="PSUM", tag="misc", name=f"vps{q}_{l}_{j}"
                        )
                        for j in range(Cr)
                    ]
                    for j in range(Cr):
                        for i in range(Cr):
                            fi = ((l * 2 + 0) * Cr + i) * Cr + j
                            nc.tensor.matmul(
                                out=v_ps[j][:],
                                lhsT=wres_sb[:, fi * P : (fi + 1) * P],
                                rhs=X[:, i * W4 : (i + 1) * W4],
                                start=(i == 0),
                                stop=(i == Cr - 1),
                            )
                    u1 = act_p.tile([P, Cr * W4], bf16, tag="u1", name=f"u1_{q}_{l}")
                    for j in range(Cr):
                        emit_silu(u1[:, j * W4 : (j + 1) * W4], v_ps[j][:])
                    w_ps = [
                        psm_p.tile(
                            [P, W4], f32, space="PSUM", tag="misc", name=f"wps{q}_{l}_{j}"
                        )
                        for j in range(Cr)
                    ]
                    for j in range(Cr):
                        for i in range(Cr):
                            fi = ((l * 2 + 1) * Cr + i) * Cr + j
                            nc.tensor.matmul(
                                out=w_ps[j][:],
                                lhsT=wres_sb[:, fi * P : (fi + 1) * P],
                                rhs=u1[:, i * W4 : (i + 1) * W4],
                                start=(i == 0),
                                stop=(i == Cr - 1),
                            )
                    Y = act_p.tile([P, Cr * W4], bf16, tag="y", name=f"Y{q}_{l}")
                    for j in range(Cr):
                        emit_silu(Y[:, j * W4 : (j + 1) * W4], w_ps[j][:])
                    Xn = act_p.tile([P, Cr * W4], bf16, tag="X", name=f"X{q}_{l + 1}")
                    nc.vector.scalar_tensor_tensor(
                        out=Xn[:],
                        in0=Y[:],
                        scalar=GAMMA[l],
                        in1=X[:],
                        op0=mybir.AluOpType.mult,
                        op1=mybir.AluOpType.add,
                    )
                    X = Xn
                o_t = out_p.tile([P, Cj * W4], f32, tag="out")
                nc.scalar.mul(out=o_t[:], in_=X[:], mul=float(C3))
                for j in range(Cj):
                    nc.sync.dma_start(
                        out=d_out[
                            :, (j * t_atom + 4 * q) * P : (j * t_atom + 4 * q + 4) * P
                        ],
                        in_=o_t[:, j * W4 : (j + 1) * W4],
                    )
    nc.compile()
    return nc


def _get_nc(t_atom, K):
    key = (t_atom, K)
    if key not in _NC_CACHE:
        _NC_CACHE[key] = _build_nc(t_atom, K)
    return _NC_CACHE[key]


# ----------------------------------------------------------------------------
# Entry point
# ----------------------------------------------------------------------------

def kernel(h, m, basis_rad, idx_atom, W_rbf, W_in, res_W1, res_W2):
    from concourse.bass_utils import run_bass_kernel_spmd

    m = np.asarray(m, dtype=np.float32)
    basis_rad = np.asarray(basis_rad, dtype=np.float32)
    idx = np.asarray(idx_atom).astype(np.int64)
    W_rbf = np.asarray(W_rbf, dtype=np.float32)
    W_in = np.asarray(W_in, dtype=np.float32)
    res_W1 = np.asarray(res_W1, dtype=np.float32)
    res_W2 = np.asarray(res_W2, dtype=np.float32)
    n_atoms = np.asarray(h).shape[0]

    layout = _pack_layout(idx, n_atoms, N_CORES, T_ATOM)
    in_maps = _build_in_maps(
        m, basis_rad, layout, W_rbf, W_in, res_W1, res_W2, N_CORES, T_ATOM
    )
    nc = _get_nc(T_ATOM, layout["K"])

    trace = os.environ.get("KERNEL_TRACE", "0") == "1"
    res = run_bass_kernel_spmd(
        nc, in_maps, core_ids=list(range(N_CORES)), trace=trace
    )
    if trace and res.exec_time_ns is not None:
        print(f"HW exec time: {res.exec_time_ns} ns", file=sys.stderr)
        kernel.last_exec_time_ns = res.exec_time_ns
    kernel.last_results = res
    return _unpack_output(res.results, layout, n_atoms, N_CORES, T_ATOM)
